# revision 1
# baseline (speedup 1.0000x reference)
"""GraphSAGE layer on 8 Trainium2 NeuronCores.

Strategy: sort edges by receiver on host; shard receivers across the 8 cores
(6250 each). Per core the segment-sum becomes local: for each 128-receiver
block, gather sender features (indirect DMA, one row per partition), build a
scaled one-hot [edge, receiver] matrix on DVE, and accumulate
features^T @ onehot into PSUM — yielding mean-aggregated features already
transposed ([feat, recv]) as lhsT for the fused Dense layer. Self features
are supplied pre-transposed; bias is added via a K=1 matmul; ReLU on DVE.
No cross-core communication is needed.
"""
import numpy as np
import concourse.bass as bass
import concourse.tile as tile
from concourse import mybir
from concourse.bass_utils import run_bass_kernel_spmd

f32 = mybir.dt.float32
i32 = mybir.dt.int32

N, D, H = 50000, 128, 256
NCORES = 8
R = N // NCORES          # receivers per core
BLK = 128                # receivers per block
NBLK = (R + BLK - 1) // BLK  # 49
RPAD = NBLK * BLK        # 6272


def _split_excess_waits(nc, max_waits=1, nop_waits=1):
    """walrus in this toolchain accepts only one sem-wait per instruction;
    hoist extra waits onto same-engine nops placed just before."""
    for bb in nc.main_func.blocks:
        insts = list(bb.instructions)
        new_list = []
        for ins in insts:
            si = ins.sync_info
            waits = list(si.on_wait) if (si is not None and si.on_wait) else []
            if len(waits) > max_waits:
                extra, keep = waits[:-max_waits], waits[-max_waits:]
                for k in range(0, len(extra), nop_waits):
                    nop = mybir.InstNoOp(
                        name=nc.get_next_instruction_name(), ins=[], outs=[]
                    )
                    nop.engine = ins.engine
                    nop.sync_info = mybir.SyncInfo(
                        on_wait=extra[k : k + nop_waits], on_update=[]
                    )
                    nc.register_instruction(nop, overwrite=True)
                    new_list.append(nop)
                si.on_wait = keep
            new_list.append(ins)
        bb.instructions[:] = new_list


_prog_cache = {}


def _build_program(T):
    NT = int(sum(T))
    nc = bass.Bass(target_bir_lowering=False)
    nodes_d = nc.dram_tensor("nodes", [N, D], f32, kind="ExternalInput")
    sndT_d = nc.dram_tensor("sndT", [128, NT], i32, kind="ExternalInput")
    ridT_d = nc.dram_tensor("ridT", [128, NT], f32, kind="ExternalInput")
    invcT_d = nc.dram_tensor("invcT", [128, NT], f32, kind="ExternalInput")
    selfT_d = nc.dram_tensor("selfT", [128, RPAD], f32, kind="ExternalInput")
    wt_d = nc.dram_tensor("wt", [128, H], f32, kind="ExternalInput")
    wb_d = nc.dram_tensor("wb", [128, H], f32, kind="ExternalInput")
    bias_d = nc.dram_tensor("bias", [1, H], f32, kind="ExternalInput")
    out_d = nc.dram_tensor("out", [R, H], f32, kind="ExternalOutput")

    with tile.TileContext(nc) as tc:
        with (
            tc.tile_pool(name="const", bufs=1) as constp,
            tc.tile_pool(name="gat", bufs=8) as gatp,
            tc.tile_pool(name="ohp", bufs=8) as ohp,
            tc.tile_pool(name="meanp", bufs=3) as meanp,
            tc.tile_pool(name="outp", bufs=3) as outp,
            tc.tile_pool(name="pmean", bufs=2, space="PSUM") as pmean,
            tc.tile_pool(name="pout", bufs=2, space="PSUM") as pout,
        ):
            iota_s = constp.tile([128, BLK], f32)
            nc.gpsimd.iota(
                iota_s[:], [[1, BLK]], channel_multiplier=0,
                allow_small_or_imprecise_dtypes=True,
            )
            ones_s = constp.tile([1, 128], f32)
            nc.vector.memset(ones_s[:], 1.0)
            wt_s = constp.tile([128, H], f32)
            nc.sync.dma_start(wt_s[:], wt_d[:])
            wb_s = constp.tile([128, H], f32)
            nc.sync.dma_start(wb_s[:], wb_d[:])
            bias_s = constp.tile([1, H], f32)
            nc.sync.dma_start(bias_s[:], bias_d[:])
            selfT_s = constp.tile([128, RPAD], f32)
            nc.sync.dma_start(selfT_s[:], selfT_d[:])
            sndT_s = constp.tile([128, NT], i32)
            nc.sync.dma_start(sndT_s[:], sndT_d[:])
            ridT_s = constp.tile([128, NT], f32)
            nc.sync.dma_start(ridT_s[:], ridT_d[:])
            invcT_s = constp.tile([128, NT], f32)
            nc.sync.dma_start(invcT_s[:], invcT_d[:])

            off = 0
            for j in range(NBLK):
                Tb = int(T[j])
                pm = pmean.tile([128, BLK], f32)
                for t in range(Tb):
                    g = gatp.tile([128, D], f32, tag="g")
                    nc.gpsimd.indirect_dma_start(
                        out=g[:],
                        out_offset=None,
                        in_=nodes_d[:],
                        in_offset=bass.IndirectOffsetOnAxis(
                            ap=sndT_s[:, off + t : off + t + 1], axis=0
                        ),
                    )
                    oh = ohp.tile([128, BLK], f32, tag="oh")
                    nc.vector.tensor_scalar(
                        out=oh[:],
                        in0=iota_s[:],
                        scalar1=ridT_s[:, off + t : off + t + 1],
                        scalar2=invcT_s[:, off + t : off + t + 1],
                        op0=mybir.AluOpType.is_equal,
                        op1=mybir.AluOpType.mult,
                    )
                    nc.tensor.matmul(
                        out=pm[:], lhsT=g[:], rhs=oh[:],
                        start=(t == 0), stop=(t == Tb - 1),
                    )
                mean_s = meanp.tile([128, BLK], f32)
                nc.vector.tensor_copy(out=mean_s[:], in_=pm[:])
                po = pout.tile([128, H], f32)
                nc.tensor.matmul(out=po[:], lhsT=mean_s[:], rhs=wt_s[:],
                                 start=True, stop=False)
                nc.tensor.matmul(out=po[:],
                                 lhsT=selfT_s[:, j * BLK : (j + 1) * BLK],
                                 rhs=wb_s[:], start=False, stop=False)
                nc.tensor.matmul(out=po[:], lhsT=ones_s[:], rhs=bias_s[:],
                                 start=False, stop=True)
                ot = outp.tile([128, H], f32)
                nc.vector.tensor_scalar_max(out=ot[:], in0=po[:], scalar1=0.0)
                bs = min(BLK, R - j * BLK)
                nc.sync.dma_start(out_d[j * BLK : j * BLK + bs, :], ot[:bs, :])
                off += Tb

    _split_excess_waits(nc)
    return nc


def kernel(**inputs):
    nodes = np.ascontiguousarray(np.asarray(inputs["nodes"], dtype=np.float32))
    senders = np.asarray(inputs["senders"]).astype(np.int64)
    receivers = np.asarray(inputs["receivers"]).astype(np.int64)
    W = np.asarray(inputs["W"], dtype=np.float32)
    b = np.asarray(inputs["b"], dtype=np.float32)

    counts = np.bincount(receivers, minlength=N).astype(np.float32)
    invc_all = (1.0 / np.maximum(counts, 1.0)).astype(np.float32)

    order = np.argsort(receivers, kind="stable")
    rs = receivers[order]
    ss = senders[order]

    core_of = rs // R
    blk_of = (rs % R) // BLK
    gblk = core_of * NBLK + blk_of
    cnt = np.bincount(gblk, minlength=NCORES * NBLK)
    T = np.maximum(
        np.ceil(cnt.reshape(NCORES, NBLK) / 128.0).astype(np.int64).max(axis=0), 1
    )
    NT = int(T.sum())
    colof = np.zeros(NBLK, np.int64)
    colof[1:] = np.cumsum(T)[:-1]
    seg_end = np.cumsum(cnt)
    seg_start = seg_end - cnt

    snd = np.zeros((NCORES, NT * 128), np.int32)
    rid = np.full((NCORES, NT * 128), -1.0, np.float32)
    invc = np.zeros((NCORES, NT * 128), np.float32)
    for c in range(NCORES):
        for j in range(NBLK):
            gi = c * NBLK + j
            n = int(cnt[gi])
            if n == 0:
                continue
            s0 = int(seg_start[gi])
            d0 = int(colof[j]) * 128
            snd[c, d0 : d0 + n] = ss[s0 : s0 + n]
            rr = rs[s0 : s0 + n]
            rid[c, d0 : d0 + n] = (rr - c * R - j * BLK).astype(np.float32)
            invc[c, d0 : d0 + n] = invc_all[rr]

    wt = np.ascontiguousarray(W[:D, :])
    wb = np.ascontiguousarray(W[D:, :])
    bias = np.ascontiguousarray(b.reshape(1, H))

    key = (NT, tuple(int(x) for x in T))
    if key not in _prog_cache:
        _prog_cache[key] = _build_program(T)
    nc = _prog_cache[key]

    in_maps = []
    for c in range(NCORES):
        selfT = np.zeros((D, RPAD), np.float32)
        selfT[:, :R] = nodes[c * R : (c + 1) * R].T
        in_maps.append(
            {
                "nodes": nodes,
                "sndT": np.ascontiguousarray(snd[c].reshape(NT, 128).T),
                "ridT": np.ascontiguousarray(rid[c].reshape(NT, 128).T),
                "invcT": np.ascontiguousarray(invc[c].reshape(NT, 128).T),
                "selfT": selfT,
                "wt": wt,
                "wb": wb,
                "bias": bias,
            }
        )

    res = run_bass_kernel_spmd(nc, in_maps, list(range(NCORES)))
    out = np.concatenate([res.results[c]["out"] for c in range(NCORES)], axis=0)
    return out.astype(np.float32)
